# revision 10
# baseline (speedup 1.0000x reference)
"""Trainium2 Bass kernel for nn_DiaSeq: batch-parallel GRU decoder forward.

Strategy
--------
Data parallel over 8 NeuronCores: batch 8192 -> 1024 per core. All weights
replicated. Each core runs: encoder MLP (h0), GRU teacher-forced scan (T=20),
per-step logits + log-softmax NLL + argmax one-hot accumulation. Host combines
per-core partial NLL sums into the scalar loss and concatenates the pred
one-hot shards.

All matmuls use a bf16 hi/lo split with 3-product accumulation in fp32 PSUM
(error ~2^-16, empirically indistinguishable from fp32 for this model), at
3 cycles/row on the PE vs 4 for native fp32.

Layout is feature-major ("transposed"): h is kept as [DEC, batch] so the
contraction dim of every matmul lands on SBUF partitions with no transposes
anywhere. Logits are produced batch-major directly by using h-chunks as the
stationary operand. The per-step embedding gather is a one-hot matmul against
a host-precomputed emb @ Wih[:EMB] table.
"""

import sys

import numpy as np
import ml_dtypes

try:
    import concourse.bass as bass
except ImportError:  # pragma: no cover
    sys.path.insert(0, "/opt/trn_rl_repo")
    import concourse.bass as bass

import concourse.bacc as bacc
import concourse.mybir as mybir
import concourse.tile as tile

# The agent image's antenv lacks axon_hooks; bass_utils imports it
# unconditionally when trace=True under axon. Provide the module, wiring in
# the boot shim's ctypes NTFF hook when available.
if "antenv.axon_hooks" not in sys.modules:
    import types
    import antenv

    _hooks_mod = types.ModuleType("antenv.axon_hooks")
    _hook_box = [None]
    _hooks_mod.get_axon_ntff_profile_hook = lambda: _hook_box[0]
    _hooks_mod.set_axon_ntff_profile_hook = lambda h: _hook_box.__setitem__(0, h)
    sys.modules["antenv.axon_hooks"] = _hooks_mod
    antenv.axon_hooks = _hooks_mod
    try:
        from trn_agent_boot.trn_boot import _ntff_profile_via_ctypes

        _hook_box[0] = _ntff_profile_via_ctypes("/opt/axon/libaxon_pjrt.so")
    except Exception:
        pass

from concourse.bass_utils import run_bass_kernel_spmd

AF = mybir.ActivationFunctionType
ALU = mybir.AluOpType
AX = mybir.AxisListType
F32 = mybir.dt.float32
BF16 = mybir.dt.bfloat16
bfloat16 = ml_dtypes.bfloat16

# Problem shapes (hardcoded per contract)
B, S, H, DEC, EMB, T = 8192, 1024, 2048, 1024, 256, 20
V, VP = 169, 256          # vocab, padded vocab
SOS, IGNORE = 166, 168
NCORES = 8
BL = B // NCORES          # 1024 batch rows per core
NB = 512                  # batch columns per scan pass
Q = BL // NB              # scan passes per core
KS, KH, KD, K3 = S // 128, H // 128, DEC // 128, (3 * DEC) // 128  # 8,16,8,24
BC = NB // 128            # batch-major 128-chunks per pass


def _split(x):
    hi = x.astype(bfloat16)
    lo = (x.astype(np.float32) - hi.astype(np.float32)).astype(bfloat16)
    return np.ascontiguousarray(hi), np.ascontiguousarray(lo)


def build_kernel(t_steps=T, q_passes=Q):
    nc = bacc.Bacc(None, target_bir_lowering=False)

    def din(name, shape, dtype=BF16):
        return nc.dram_tensor(name, list(shape), dtype, kind="ExternalInput")

    # --- external inputs (per core) ---
    d_sT_h = din("sT_h", (S, BL)); d_sT_l = din("sT_l", (S, BL))
    d_w1_h = din("w1_h", (S, H)); d_w1_l = din("w1_l", (S, H))
    d_w2_h = din("w2_h", (H, DEC)); d_w2_l = din("w2_l", (H, DEC))
    d_wi_h = din("wi_h", (DEC, 3 * DEC)); d_wi_l = din("wi_l", (DEC, 3 * DEC))
    d_wh_h = din("wh_h", (DEC, 3 * DEC)); d_wh_l = din("wh_l", (DEC, 3 * DEC))
    d_wp_h = din("wp_h", (DEC, V)); d_wp_l = din("wp_l", (DEC, V))
    d_ep_h = din("ep_h", (VP, 3 * DEC)); d_ep_l = din("ep_l", (VP, 3 * DEC))
    d_ohin = din("ohin", (T, VP, BL))          # one-hot of input token, bf16
    d_tgt = din("tgt", (T, BL, V))             # one-hot of target, bf16 batch-major
    d_b1 = din("b1", (128, KH), F32)           # biases, chunk-major [128, nchunk]
    d_b2 = din("b2", (128, KD), F32)
    d_brz = din("brz", (128, 16), F32)         # (bih+bhh)[:2048] chunks
    d_bhn = din("bhn", (128, KD), F32)         # bhh n-part
    d_bin = din("bin", (128, KD), F32)         # bih n-part

    # --- outputs ---
    d_pred = nc.dram_tensor("pred", [Q, BC, 128, V], F32, kind="ExternalOutput")
    d_nll = nc.dram_tensor("nll", [Q, BC, 128, t_steps], F32, kind="ExternalOutput")

    # --- internal scratch ---
    d_h0p = nc.dram_tensor("h0p_scr", [K3, 128, BL], F32, kind="Internal")
    d_h0h = nc.dram_tensor("h0h_scr", [KD, 128, BL], BF16, kind="Internal")
    d_h0l = nc.dram_tensor("h0l_scr", [KD, 128, BL], BF16, kind="Internal")

    def r3(d, p=128):
        return d.ap().rearrange("(c p) m -> p c m", p=p)

    with tile.TileContext(nc) as tc:
        def accum3(psum, wh, wl, xh, xl, kc, mlo, mhi, nlo, nhi):
            """psum += sum_k W[k]^T X[k] (bf16 hi/lo 3-product accumulation)."""
            n = 0
            total = kc * 3
            for k in range(kc):
                w_h = wh[:, k, mlo:mhi]
                x_h = xh[:, k, nlo:nhi]
                nc.tensor.matmul(psum, w_h, x_h, start=(n == 0), stop=(n == total - 1)); n += 1
                nc.tensor.matmul(psum, w_h, xl[:, k, nlo:nhi], start=False, stop=(n == total - 1)); n += 1
                nc.tensor.matmul(psum, wl[:, k, mlo:mhi], x_h, start=False, stop=(n == total - 1)); n += 1

        def split_store(hf, dst_h, dst_l, sl):
            """Write fp32 tile hf into bf16 hi/lo pair at free-slice sl."""
            nc.vector.tensor_copy(dst_h[:, sl[0], sl[1]], hf[:])
            nc.vector.tensor_tensor(dst_l[:, sl[0], sl[1]], hf[:], dst_h[:, sl[0], sl[1]], ALU.subtract)

        # ============== Phase E: encoder ==============
        with tc.tile_pool(name="ph1", bufs=1) as ph1:
            h1_h = ph1.tile([128, KH, BL], BF16, tag="h1_h")
            h1_l = ph1.tile([128, KH, BL], BF16, tag="h1_l")
            b1 = ph1.tile([128, KH], F32, tag="b1")
            b2 = ph1.tile([128, KD], F32, tag="b2")
            nc.sync.dma_start(b1[:], d_b1.ap()); nc.sync.dma_start(b2[:], d_b2.ap())

            with (
                tc.tile_pool(name="enc0", bufs=1) as enc0,
                tc.tile_pool(name="etmp", bufs=3) as etmp,
                tc.tile_pool(name="eps", bufs=8, space="PSUM") as eps,
            ):
                sT_h = enc0.tile([128, KS, BL], BF16, tag="sT_h")
                sT_l = enc0.tile([128, KS, BL], BF16, tag="sT_l")
                w1_h = enc0.tile([128, KS, H], BF16, tag="w1_h")
                w1_l = enc0.tile([128, KS, H], BF16, tag="w1_l")
                nc.sync.dma_start(sT_h[:], r3(d_sT_h)); nc.sync.dma_start(sT_l[:], r3(d_sT_l))
                nc.sync.dma_start(w1_h[:], r3(d_w1_h)); nc.sync.dma_start(w1_l[:], r3(d_w1_l))

                # h1 = relu(W1^T sT + b1), feature-major [H, BL]
                for m in range(KH):
                    for nb in range(BL // 512):
                        ps = eps.tile([128, 512], F32, tag="ps")
                        accum3(ps[:], w1_h, w1_l, sT_h, sT_l, KS, m * 128, (m + 1) * 128, nb * 512, (nb + 1) * 512)
                        hf = etmp.tile([128, 512], F32, tag="ehf")
                        nc.scalar.activation(hf[:], ps[:], AF.Relu, bias=b1[:, m : m + 1])
                        split_store(hf, h1_h, h1_l, (m, slice(nb * 512, (nb + 1) * 512)))

            with tc.tile_pool(name="ph0", bufs=1) as ph0:
                h0_h = ph0.tile([128, KD, BL], BF16, tag="h0_h")
                h0_l = ph0.tile([128, KD, BL], BF16, tag="h0_l")
                with (
                    tc.tile_pool(name="enc2", bufs=1) as enc2,
                    tc.tile_pool(name="etmp2", bufs=3) as etmp2,
                    tc.tile_pool(name="eps2", bufs=8, space="PSUM") as eps2,
                ):
                    w2_h = enc2.tile([128, KH, DEC], BF16, tag="w2_h")
                    w2_l = enc2.tile([128, KH, DEC], BF16, tag="w2_l")
                    nc.sync.dma_start(w2_h[:], r3(d_w2_h)); nc.sync.dma_start(w2_l[:], r3(d_w2_l))
                    # h0 = W2^T h1 + b2
                    for m in range(KD):
                        for nb in range(BL // 512):
                            ps = eps2.tile([128, 512], F32, tag="ps")
                            accum3(ps[:], w2_h, w2_l, h1_h, h1_l, KH, m * 128, (m + 1) * 128, nb * 512, (nb + 1) * 512)
                            hf = etmp2.tile([128, 512], F32, tag="ehf2")
                            nc.scalar.activation(hf[:], ps[:], AF.Identity, bias=b2[:, m : m + 1])
                            split_store(hf, h0_h, h0_l, (m, slice(nb * 512, (nb + 1) * 512)))
                    nc.sync.dma_start(d_h0h.ap().rearrange("c p m -> p c m"), h0_h[:])
                    nc.sync.dma_start(d_h0l.ap().rearrange("c p m -> p c m"), h0_l[:])

                with (
                    tc.tile_pool(name="enc3", bufs=1) as enc3,
                    tc.tile_pool(name="etmp3", bufs=4) as etmp3,
                    tc.tile_pool(name="eps3", bufs=8, space="PSUM") as eps3,
                ):
                    wi_h = enc3.tile([128, KD, 3 * DEC], BF16, tag="wi_h")
                    wi_l = enc3.tile([128, KD, 3 * DEC], BF16, tag="wi_l")
                    nc.sync.dma_start(wi_h[:], r3(d_wi_h)); nc.sync.dma_start(wi_l[:], r3(d_wi_l))
                    # h0p = Wih[EMB:]^T h0  (feature-major [3DEC, BL], fp32) -> DRAM
                    for m in range(K3):
                        for nb in range(BL // 512):
                            ps = eps3.tile([128, 512], F32, tag="ps")
                            accum3(ps[:], wi_h, wi_l, h0_h, h0_l, KD, m * 128, (m + 1) * 128, nb * 512, (nb + 1) * 512)
                            st = etmp3.tile([128, 512], F32, tag="est")
                            nc.vector.tensor_copy(st[:], ps[:])
                            nc.sync.dma_start(d_h0p.ap()[m][:, nb * 512 : (nb + 1) * 512], st[:])

        # ============== Phase S: GRU scan ==============
        with (
            tc.tile_pool(name="sw", bufs=1) as sw,
            tc.tile_pool(name="sh", bufs=1) as sh,
            tc.tile_pool(name="sstream", bufs=4) as sstream,
            tc.tile_pool(name="sio", bufs=2) as sio,
            tc.tile_pool(name="stmp", bufs=2) as stmp,
            tc.tile_pool(name="sgate", bufs=3) as sgate,
            tc.tile_pool(name="sscr", bufs=1) as sscr,
            tc.tile_pool(name="scol", bufs=3) as scol,
            tc.tile_pool(name="sps", bufs=8, space="PSUM") as sps,
        ):
            wh_h = sw.tile([128, KD, 3 * DEC], BF16, tag="wh_h")
            wh_l = sw.tile([128, KD, 3 * DEC], BF16, tag="wh_l")
            ep_h = sw.tile([128, 2, 3 * DEC], BF16, tag="ep_h")
            ep_l = sw.tile([128, 2, 3 * DEC], BF16, tag="ep_l")
            wp_h = sw.tile([128, KD, V], BF16, tag="wp_h")
            wp_l = sw.tile([128, KD, V], BF16, tag="wp_l")
            brz = sw.tile([128, 16], F32, tag="brz")
            bhn = sw.tile([128, KD], F32, tag="bhn")
            bin_ = sw.tile([128, KD], F32, tag="bin")
            nc.sync.dma_start(wh_h[:], r3(d_wh_h)); nc.sync.dma_start(wh_l[:], r3(d_wh_l))
            nc.sync.dma_start(ep_h[:], r3(d_ep_h)); nc.sync.dma_start(ep_l[:], r3(d_ep_l))
            nc.sync.dma_start(wp_h[:], r3(d_wp_h)); nc.sync.dma_start(wp_l[:], r3(d_wp_l))
            nc.sync.dma_start(brz[:], d_brz.ap())
            nc.sync.dma_start(bhn[:], d_bhn.ap()); nc.sync.dma_start(bin_[:], d_bin.ap())

            for q in range(q_passes):
                ncol = NB
                hA_h = sh.tile([128, KD, NB], BF16, tag="hA_h")
                hA_l = sh.tile([128, KD, NB], BF16, tag="hA_l")
                hB_h = sh.tile([128, KD, NB], BF16, tag="hB_h")
                hB_l = sh.tile([128, KD, NB], BF16, tag="hB_l")
                pred_acc = sh.tile([128, BC, V], F32, tag="pred_acc")
                nll_acc = sh.tile([128, BC, t_steps], F32, tag="nll_acc")
                nc.vector.memset(pred_acc[:], 0.0)
                nc.sync.dma_start(hA_h[:], d_h0h.ap().rearrange("c p m -> p c m")[:, :, q * NB : (q + 1) * NB])
                nc.sync.dma_start(hA_l[:], d_h0l.ap().rearrange("c p m -> p c m")[:, :, q * NB : (q + 1) * NB])

                cur_h, cur_l, nxt_h, nxt_l = hA_h, hA_l, hB_h, hB_l
                for t in range(t_steps):
                    oh = sio.tile([128, 2, NB], BF16, tag="oh")
                    nc.sync.dma_start(oh[:], d_ohin.ap()[t][:, q * NB : (q + 1) * NB].rearrange("(c p) n -> p c n", p=128))
                    tg = sio.tile([128, BC, V], BF16, tag="tg")
                    nc.sync.dma_start(tg[:], d_tgt.ap()[t][q * NB : (q + 1) * NB, :].rearrange("(c p) v -> p c v", p=128))

                    for j in range(KD):
                        # --- r and z gates ---
                        gates = []
                        for g in range(2):
                            m = g * KD + j
                            ps = sps.tile([128, NB], F32, tag="ps")
                            n_mm = 2 * 2 + KD * 3
                            i = 0
                            for k in range(2):
                                nc.tensor.matmul(ps[:], ep_h[:, k, m * 128 : (m + 1) * 128], oh[:, k, :], start=(i == 0), stop=False); i += 1
                                nc.tensor.matmul(ps[:], ep_l[:, k, m * 128 : (m + 1) * 128], oh[:, k, :], start=False, stop=False); i += 1
                            for k in range(KD):
                                w_h = wh_h[:, k, m * 128 : (m + 1) * 128]
                                nc.tensor.matmul(ps[:], w_h, cur_h[:, k, :], start=False, stop=False); i += 1
                                nc.tensor.matmul(ps[:], w_h, cur_l[:, k, :], start=False, stop=False); i += 1
                                nc.tensor.matmul(ps[:], wh_l[:, k, m * 128 : (m + 1) * 128], cur_h[:, k, :], start=False, stop=(i == n_mm - 1)); i += 1
                            h0p = sstream.tile([128, NB], F32, tag="h0p")
                            nc.sync.dma_start(h0p[:], d_h0p.ap()[m][:, q * NB : (q + 1) * NB])
                            nc.vector.tensor_tensor(ps[:], ps[:], h0p[:], ALU.add)
                            gate = sgate.tile([128, NB], F32, tag="gate")
                            nc.scalar.activation(gate[:], ps[:], AF.Sigmoid, bias=brz[:, m : m + 1])
                            gates.append(gate)
                        r_j, z_j = gates

                        # --- n gate ---
                        m = 2 * KD + j
                        ph = sps.tile([128, NB], F32, tag="ps")
                        i = 0
                        for k in range(KD):
                            w_h = wh_h[:, k, m * 128 : (m + 1) * 128]
                            nc.tensor.matmul(ph[:], w_h, cur_h[:, k, :], start=(i == 0), stop=False); i += 1
                            nc.tensor.matmul(ph[:], w_h, cur_l[:, k, :], start=False, stop=False); i += 1
                            nc.tensor.matmul(ph[:], wh_l[:, k, m * 128 : (m + 1) * 128], cur_h[:, k, :], start=False, stop=(i == KD * 3 - 1)); i += 1
                        hn = stmp.tile([128, NB], F32, tag="hn")
                        nc.scalar.activation(hn[:], ph[:], AF.Identity, bias=bhn[:, j : j + 1])

                        pg = sps.tile([128, NB], F32, tag="ps")
                        for k in range(2):
                            nc.tensor.matmul(pg[:], ep_h[:, k, m * 128 : (m + 1) * 128], oh[:, k, :], start=(k == 0), stop=False)
                            nc.tensor.matmul(pg[:], ep_l[:, k, m * 128 : (m + 1) * 128], oh[:, k, :], start=False, stop=(k == 1))
                        h0pn = sstream.tile([128, NB], F32, tag="h0p")
                        nc.sync.dma_start(h0pn[:], d_h0p.ap()[m][:, q * NB : (q + 1) * NB])

                        t1 = stmp.tile([128, NB], F32, tag="t1")
                        nc.vector.tensor_tensor(t1[:], r_j[:], hn[:], ALU.mult)
                        nc.vector.tensor_tensor(pg[:], pg[:], t1[:], ALU.add)
                        nc.vector.tensor_tensor(pg[:], pg[:], h0pn[:], ALU.add)
                        n_j = stmp.tile([128, NB], F32, tag="n")
                        nc.scalar.activation(n_j[:], pg[:], AF.Tanh, bias=bin_[:, j : j + 1])

                        # --- h update: h' = n + z * (h - n) ---
                        d1 = stmp.tile([128, NB], F32, tag="d1")
                        nc.vector.tensor_tensor(d1[:], cur_h[:, j, :], n_j[:], ALU.subtract)
                        nc.vector.tensor_tensor(d1[:], d1[:], cur_l[:, j, :], ALU.add)
                        nc.vector.tensor_tensor(d1[:], d1[:], z_j[:], ALU.mult)
                        hf = stmp.tile([128, NB], F32, tag="hf")
                        nc.vector.tensor_tensor(hf[:], d1[:], n_j[:], ALU.add)
                        nc.scalar.activation(nxt_h[:, j, :], hf[:], AF.Copy)
                        nc.vector.tensor_tensor(nxt_l[:, j, :], hf[:], nxt_h[:, j, :], ALU.subtract)

                    # --- logits, nll, pred (batch-major), uses h_new ---
                    for bc in range(BC):
                        pl = sps.tile([128, V], F32, tag="ps")
                        i = 0
                        for k in range(KD):
                            l_h = nxt_h[:, k, bc * 128 : (bc + 1) * 128]
                            l_l = nxt_l[:, k, bc * 128 : (bc + 1) * 128]
                            nc.tensor.matmul(pl[:], l_h, wp_h[:, k, :], start=(i == 0), stop=False); i += 1
                            nc.tensor.matmul(pl[:], l_h, wp_l[:, k, :], start=False, stop=False); i += 1
                            nc.tensor.matmul(pl[:], l_l, wp_h[:, k, :], start=False, stop=(i == KD * 3 - 1)); i += 1
                        se = scol.tile([128, 1], F32, tag="se")
                        escr = sscr.tile([128, V], F32, tag="escr")
                        nc.scalar.activation(escr[:], pl[:], AF.Exp, accum_out=se[:])
                        ln = scol.tile([128, 1], F32, tag="ln")
                        nc.scalar.activation(ln[:], se[:], AF.Ln)
                        tgc = scol.tile([128, 1], F32, tag="tgc")
                        tscr = sscr.tile([128, V], F32, tag="tscr")
                        nc.vector.scalar_tensor_tensor(tscr[:], pl[:], 1.0, tg[:, bc, :], ALU.mult, ALU.mult, accum_out=tgc[:])
                        nc.vector.tensor_scalar(nll_acc[:, bc, t : t + 1], ln[:], tgc[:], None, ALU.subtract)
                        mx = scol.tile([128, 1], F32, tag="mx")
                        nc.vector.tensor_reduce(mx[:], pl[:], AX.X, ALU.max)
                        pscr = sscr.tile([128, V], F32, tag="pscr")
                        nc.vector.tensor_scalar(pscr[:], pl[:], mx[:], None, ALU.is_ge)
                        nc.vector.tensor_tensor(pred_acc[:, bc, :], pred_acc[:, bc, :], pscr[:], ALU.max)

                    cur_h, cur_l, nxt_h, nxt_l = nxt_h, nxt_l, cur_h, cur_l

                nc.sync.dma_start(d_pred.ap()[q].rearrange("c p v -> p c v"), pred_acc[:])
                nc.sync.dma_start(d_nll.ap()[q].rearrange("c p t -> p c t"), nll_acc[:])

    nc.compile()
    return nc


_NC_CACHE = {}


def kernel(s, a_target_seq, W1, b1, W2, b2, emb, Wih, Whh, bih, bhh, Wp, bp):
    s = np.asarray(s, np.float32); a = np.asarray(a_target_seq, np.int32)
    W1 = np.asarray(W1, np.float32); W2 = np.asarray(W2, np.float32)
    Wih = np.asarray(Wih, np.float32); Whh = np.asarray(Whh, np.float32)
    Wp = np.asarray(Wp, np.float32); emb = np.asarray(emb, np.float32)
    b1 = np.asarray(b1, np.float32); b2 = np.asarray(b2, np.float32)
    bih = np.asarray(bih, np.float32); bhh = np.asarray(bhh, np.float32)
    bp = np.asarray(bp, np.float32)
    assert np.abs(bp).max() == 0.0, "nonzero bp not wired"

    # ---- host weight preprocessing (layout + bf16 hi/lo splits) ----
    w1h, w1l = _split(W1)
    w2h, w2l = _split(W2)
    wih, wil = _split(Wih[EMB:])
    whh, whl = _split(Whh)
    wph, wpl = _split(Wp)
    ep = np.zeros((VP, 3 * DEC), np.float64)
    ep[:V] = emb.astype(np.float64) @ Wih[:EMB].astype(np.float64)
    eph, epl = _split(ep.astype(np.float32))

    def chunk_bias(v, kc):  # [kc*128] -> [128, kc]
        return np.ascontiguousarray(v.reshape(kc, 128).T.astype(np.float32))

    common = {
        "w1_h": w1h, "w1_l": w1l, "w2_h": w2h, "w2_l": w2l,
        "wi_h": wih, "wi_l": wil, "wh_h": whh, "wh_l": whl,
        "wp_h": wph, "wp_l": wpl, "ep_h": eph, "ep_l": epl,
        "b1": chunk_bias(b1, KH), "b2": chunk_bias(b2, KD),
        "brz": chunk_bias((bih + bhh)[: 2 * DEC], 16),
        "bhn": chunk_bias(bhh[2 * DEC :], KD), "bin": chunk_bias(bih[2 * DEC :], KD),
    }

    key = "full"
    if key not in _NC_CACHE:
        _NC_CACHE[key] = build_kernel()
    nc = _NC_CACHE[key]

    in_maps = []
    for c in range(NCORES):
        sl = slice(c * BL, (c + 1) * BL)
        ac = a[sl]                                  # [BL, T]
        in_tok = np.concatenate([np.full((BL, 1), SOS, np.int32), ac[:, :-1]], 1)
        ohin = np.zeros((T, VP, BL), bfloat16)
        tgt = np.zeros((T, BL, V), bfloat16)
        bi = np.arange(BL)
        for t in range(T):
            ohin[t, in_tok[:, t], bi] = 1.0
            tgt[t, bi, ac[:, t]] = 1.0
        sh, sl_ = _split(np.ascontiguousarray(s[sl].T))
        m = dict(common)
        m.update({"sT_h": sh, "sT_l": sl_, "ohin": ohin, "tgt": tgt})
        in_maps.append(m)

    res = run_bass_kernel_spmd(nc, in_maps, core_ids=list(range(NCORES)))
    global LAST_EXEC_NS
    LAST_EXEC_NS = res.exec_time_ns

    preds, nlls = [], []
    for c in range(NCORES):
        pred = res.results[c]["pred"]               # [Q, BC, 128, V]
        nll = res.results[c]["nll"]                 # [Q, BC, 128, T]
        preds.append(pred.reshape(BL, V))
        nlls.append(nll.reshape(BL, T))
    pred_full = np.concatenate(preds, 0)[:, : V - 3].astype(np.float32)
    nll_full = np.concatenate(nlls, 0)              # [B, T]
    mask = (a != IGNORE)
    denom = max(mask.sum(), 1)
    loss = np.float32(np.sum(nll_full, where=mask, dtype=np.float64) / denom)
    return np.array(loss, np.float32), pred_full
